# revision 3
# baseline (speedup 1.0000x reference)
"""CIN (Compressed Interaction Network) kernel for Trainium2, 8 NeuronCores.

Problem: x (2048, 39, 16) f32; 3 CIN layers with W_i (200, 39, prev):
    z[b,o,d] = sum_{f,g} W[o,f,g] * x0[b,f,d] * h[b,g,d] + bias[o]
    h' = relu(z);  output = sum_d concat([h1,h2,h3], ch) -> (2048, 600)

Strategy (data-parallel over batch, 8 cores, 256 batch rows each):
  Per core, columns n = (b_local, d), N = 256*16 = 4096, in 8 n-tiles of 512.
  For each layer, z[:, n] = sum_f (W[:, f, :] @ (h ⊙ bcast(x0[f, :]))) — the
  Khatri-Rao factor V_f = h ⊙ x0[f] is built on the Vector engine (fp16
  tensor_tensor) against a partition-replicated x0 tile (built once per
  n-tile via broadcast DMA), and consumed by per-f fp16 matmuls accumulated
  in PSUM over (f, g-chunks). Bias+relu fused on the Scalar engine; the
  d-sum runs on the Vector engine; the output transpose happens on host.
"""
import numpy as np

import concourse.bacc as bacc
import concourse.mybir as mybir
import concourse.tile as tile
from concourse.bass_utils import run_bass_kernel_spmd

B, F0, D = 2048, 39, 16
C = 200                      # cross size per layer
NCORES = 8
BC = B // NCORES             # 256 batch rows per core
N = BC * D                   # 4096 columns per core
NT = 512                     # n-tile width
T = N // NT                  # 8 n-tiles
BT = NT // D                 # 32 batch rows per n-tile
OA, OB = 128, C - 128        # output-channel chunks (128 + 72)
F16 = mybir.dt.float16
F32 = mybir.dt.float32


def _build_nc():
    nc = bacc.Bacc(None, target_bir_lowering=False)

    NC0 = (F0 + 1) // 2  # 20 packed L0 chunks (2 f's per 128-row chunk)
    x0_d = nc.dram_tensor("x0", [F0, N], F16, kind="ExternalInput")
    w0_d = nc.dram_tensor("w0", [128, NC0 * C], F16, kind="ExternalInput")
    w1a_d = nc.dram_tensor("w1a", [OA, F0 * C], F16, kind="ExternalInput")
    w1b_d = nc.dram_tensor("w1b", [OB, F0 * C], F16, kind="ExternalInput")
    w2a_d = nc.dram_tensor("w2a", [OA, F0 * C], F16, kind="ExternalInput")
    w2b_d = nc.dram_tensor("w2b", [OB, F0 * C], F16, kind="ExternalInput")
    b_d = nc.dram_tensor("b", [3 * C, 1], F32, kind="ExternalInput")
    out_d = nc.dram_tensor("out3", [3, C, BC], F32, kind="ExternalOutput")

    with tile.TileContext(nc) as tc:
        with (
            tc.tile_pool(name="wp", bufs=1) as wp,
            tc.tile_pool(name="bc", bufs=2) as bcp,
            tc.tile_pool(name="hp", bufs=2) as hp,
            tc.tile_pool(name="vp", bufs=4) as vp,
            tc.tile_pool(name="ps", bufs=2, space="PSUM") as ps,
        ):
            # --- static loads -------------------------------------------------
            x0 = wp.tile([F0, N], F16)
            nc.sync.dma_start(out=x0[:], in_=x0_d[:])
            # xpad: x0 rows replicated at partition bases 0 and 64, zero pad
            xpad = wp.tile([128, N], F16)
            nc.vector.memset(xpad[:], 0.0)
            nc.sync.dma_start(out=xpad[0:F0, :], in_=x0_d[:])
            nc.sync.dma_start(out=xpad[64:64 + F0, :], in_=x0_d[:])
            w0 = wp.tile([128, NC0 * C], F16)
            nc.sync.dma_start(out=w0[:], in_=w0_d[:])
            w1a = wp.tile([OA, F0 * C], F16)
            nc.sync.dma_start(out=w1a[:], in_=w1a_d[:])
            w1b = wp.tile([OB, F0 * C], F16)
            nc.sync.dma_start(out=w1b[:], in_=w1b_d[:])
            w2a = wp.tile([OA, F0 * C], F16)
            nc.sync.dma_start(out=w2a[:], in_=w2a_d[:])
            w2b = wp.tile([OB, F0 * C], F16)
            nc.sync.dma_start(out=w2b[:], in_=w2b_d[:])
            biases = []
            for l in range(3):
                ba = wp.tile([OA, 1], F32, tag=f"b{l}a")
                bb = wp.tile([OB, 1], F32, tag=f"b{l}b")
                nc.sync.dma_start(out=ba[:], in_=b_d[l * C:l * C + OA, :])
                nc.sync.dma_start(out=bb[:], in_=b_d[l * C + OA:(l + 1) * C, :])
                biases.append((ba, bb))
            # per-layer output accumulators [o, b_local]
            outs = []
            for l in range(3):
                oa = wp.tile([OA, BC], F32, tag=f"o{l}a")
                ob = wp.tile([OB, BC], F32, tag=f"o{l}b")
                outs.append((oa, ob))

            for t in range(T):
                ns = slice(t * NT, (t + 1) * NT)
                # replicated x0 rows: xb[p, f*NT + j] = x0[f, t*NT + j]
                xb = bcp.tile([128, F0 * NT], F16, tag="xb")
                for f in range(F0):
                    nc.sync.dma_start(
                        out=xb[:, f * NT:(f + 1) * NT],
                        in_=x0[f:f + 1, ns].unsqueeze(1).broadcast_to((1, 128, NT)),
                    )

                ha, hb = None, None
                for l, (wa, wb, ga, gb) in enumerate((
                    (w0, None, F0, 0),
                    (w1a, w1b, OA, OB),
                    (w2a, w2b, OA, OB),
                )):
                    pa = ps.tile([OA, NT], F32, tag="pa")
                    pb = ps.tile([OB, NT], F32, tag="pb")
                    for f in range(F0):
                        fs = slice(f * NT, (f + 1) * NT)
                        os_a = slice(f * C, f * C + OA)
                        os_b = slice(f * C + OA, (f + 1) * C)
                        first, last = f == 0, f == F0 - 1
                        if l == 0:
                            v = vp.tile([F0, NT], F16, tag="v0")
                            nc.vector.tensor_tensor(
                                out=v[:], in0=x0[:, ns], in1=xb[0:F0, fs],
                                op=mybir.AluOpType.mult)
                            nc.tensor.matmul(pa[:], wa[:, os_a], v[:],
                                             start=first, stop=last)
                            nc.tensor.matmul(pb[:], wa[:, os_b], v[:],
                                             start=first, stop=last)
                        else:
                            va = vp.tile([ga, NT], F16, tag="va")
                            vb = vp.tile([gb, NT], F16, tag="vb")
                            nc.vector.tensor_tensor(
                                out=va[:], in0=ha[:], in1=xb[0:ga, fs],
                                op=mybir.AluOpType.mult)
                            nc.vector.tensor_tensor(
                                out=vb[:], in0=hb[:], in1=xb[0:gb, fs],
                                op=mybir.AluOpType.mult)
                            nc.tensor.matmul(pa[:], wa[:, os_a], va[:],
                                             start=first, stop=False)
                            nc.tensor.matmul(pa[:], wb[:, os_a], vb[:],
                                             start=False, stop=last)
                            nc.tensor.matmul(pb[:], wa[:, os_b], va[:],
                                             start=first, stop=False)
                            nc.tensor.matmul(pb[:], wb[:, os_b], vb[:],
                                             start=False, stop=last)
                    # bias + relu -> fp16 h
                    ba, bb = biases[l]
                    ha = hp.tile([OA, NT], F16, tag="ha")
                    hb = hp.tile([OB, NT], F16, tag="hb")
                    nc.scalar.activation(ha[:], pa[:],
                                         mybir.ActivationFunctionType.Relu,
                                         bias=ba[:])
                    nc.scalar.activation(hb[:], pb[:],
                                         mybir.ActivationFunctionType.Relu,
                                         bias=bb[:])
                    # d-sum into the per-layer accumulator column block
                    oa, ob = outs[l]
                    bs = slice(t * BT, (t + 1) * BT)
                    nc.vector.tensor_reduce(
                        out=oa[:, bs],
                        in_=ha[:].rearrange("p (b d) -> p b d", d=D),
                        axis=mybir.AxisListType.X, op=mybir.AluOpType.add)
                    nc.vector.tensor_reduce(
                        out=ob[:, bs],
                        in_=hb[:].rearrange("p (b d) -> p b d", d=D),
                        axis=mybir.AxisListType.X, op=mybir.AluOpType.add)

            for l in range(3):
                oa, ob = outs[l]
                nc.sync.dma_start(out=out_d[l, 0:OA, :], in_=oa[:])
                nc.sync.dma_start(out=out_d[l, OA:C, :], in_=ob[:])

    nc.compile()
    return nc


_NC_CACHE = None


def _get_nc():
    global _NC_CACHE
    if _NC_CACHE is None:
        _NC_CACHE = _build_nc()
    return _NC_CACHE


def _prep_weights(W0, W1, W2, b0, b1, b2):
    # lhsT layout: w[g, f*C + o] = W[o, f, g]
    def lay(W):
        return np.ascontiguousarray(
            W.transpose(2, 1, 0).reshape(W.shape[2], F0 * C)).astype(np.float16)

    w0 = lay(np.asarray(W0))          # [39, 7800]
    w1 = lay(np.asarray(W1))          # [200, 7800]
    w2 = lay(np.asarray(W2))
    b = np.concatenate([np.asarray(b0), np.asarray(b1), np.asarray(b2)])
    return {
        "w0": w0,
        "w1a": np.ascontiguousarray(w1[:OA]), "w1b": np.ascontiguousarray(w1[OA:]),
        "w2a": np.ascontiguousarray(w2[:OA]), "w2b": np.ascontiguousarray(w2[OA:]),
        "b": b.astype(np.float32).reshape(3 * C, 1),
    }


def kernel(x, W0, b0, W1, b1, W2, b2):
    x = np.asarray(x)
    assert x.shape == (B, F0, D), x.shape
    nc = _get_nc()
    shared = _prep_weights(W0, W1, W2, b0, b1, b2)

    in_maps = []
    for c in range(NCORES):
        xc = x[c * BC:(c + 1) * BC]                      # [256, 39, 16]
        x0c = np.ascontiguousarray(
            xc.transpose(1, 0, 2).reshape(F0, N)).astype(np.float16)
        in_maps.append({"x0": x0c, **shared})

    res = run_bass_kernel_spmd(nc, in_maps, list(range(NCORES)))

    out = np.empty((B, 3 * C), dtype=np.float32)
    for c in range(NCORES):
        o3 = res.results[c]["out3"]                      # [3, 200, 256]
        out[c * BC:(c + 1) * BC] = o3.transpose(2, 0, 1).reshape(BC, 3 * C)
    return out
